# revision 1
# baseline (speedup 1.0000x reference)
"""Trainium2 Bass kernel for nn_AttentionOperation_32521492365427.

kernel(**inputs) -> np.ndarray, full shapes:
  query/key/value: [8, 8, 64, 1024] f32; gamma_sim/beta_sim: [8];
  gamma_val/beta_val: [512]; output: [8, 512, 1024] f32.

Sharded by HEAD across the 8 NeuronCores (one head per core): both
BatchNorms then have core-local statistics, so there are no collectives.

Per-core math:
 - softmax is shift-invariant => the sim-BN reduces to one per-head scale
   s = gamma_sim / sqrt(var(logits) + EPS); beta/mean drop out.
 - sumsq(logits_b) = sum(Gq_b * Gk_b) over 64x64 Gram matrices
   Gq = Q1^T Q1 (ones column appended) => logits variance without a
   stats pass over the 8.4M logits.
 - softmax denominators come free as row 64 of the PV matmul by appending
   a ones column to V^T (the stationary operand).
 - val-BN affine + exact (erf) gelu fuse into a single ACT pass.
"""

import os
import sys

sys.path.insert(0, "/opt/trn_rl_repo")

from contextlib import ExitStack

import numpy as np

import concourse.bacc as bacc
import concourse.bass as bass  # noqa: F401
import concourse.tile as tile
from concourse import mybir

F32 = mybir.dt.float32
BF16 = mybir.dt.bfloat16
I32 = mybir.dt.int32
AF = mybir.ActivationFunctionType
OP = mybir.AluOpType

EPS = 1e-3
NB = 8
D = 64
C = 64
L = 1024
M = 1024
NCH = M // 128
NLM = float(NB * L * M)
MAGIC = 0x5F3759DF


def _newton_rsqrt(nc, x, y, t, magic_i32, iters=3):
    """y = 1/sqrt(x) entirely on DVE (bit-trick seed + Newton iters)."""
    xi = x.bitcast(I32)
    yi = y.bitcast(I32)
    nc.vector.tensor_scalar(
        out=yi, in0=xi, scalar1=1, scalar2=None, op0=OP.arith_shift_right
    )
    nc.vector.tensor_tensor(out=yi, in0=magic_i32, in1=yi, op=OP.subtract)
    for _ in range(iters):
        nc.vector.tensor_mul(t, y, y)
        nc.vector.tensor_mul(t, t, x)
        nc.vector.tensor_scalar(
            out=t, in0=t, scalar1=-0.5, scalar2=1.5, op0=OP.mult, op1=OP.add
        )
        nc.vector.tensor_mul(y, y, t)


def build_nc(debug: bool = False):
    nc = bacc.Bacc("TRN2", target_bir_lowering=False, debug=debug)

    q2_d = nc.dram_tensor("q2", [NB // 2, 128, L], F32, kind="ExternalInput")
    k2_d = nc.dram_tensor("k2", [NB // 2, 128, L], F32, kind="ExternalInput")
    qt1_d = nc.dram_tensor("qt1", [NB, 128, NCH, 65], F32, kind="ExternalInput")
    kt1_d = nc.dram_tensor("kt1", [NB, 128, NCH, 65], F32, kind="ExternalInput")
    vt1_d = nc.dram_tensor("vt1", [NB, 128, NCH, 65], F32, kind="ExternalInput")
    gsim_d = nc.dram_tensor("g_sim", [1, 1], F32, kind="ExternalInput")
    gval_d = nc.dram_tensor("gamma_val", [C, 1], F32, kind="ExternalInput")
    bval_d = nc.dram_tensor("beta_val", [C, 1], F32, kind="ExternalInput")
    out_d = nc.dram_tensor("out", [NB, C, L], F32, kind="ExternalOutput")

    with tile.TileContext(nc) as tc, ExitStack() as ctx:
        const_p = ctx.enter_context(tc.tile_pool(name="const", bufs=1))
        vt_p = ctx.enter_context(tc.tile_pool(name="vt", bufs=3))
        q2_p = ctx.enter_context(tc.tile_pool(name="q2", bufs=2))
        pt_p = ctx.enter_context(tc.tile_pool(name="pt", bufs=10))
        rv_p = ctx.enter_context(tc.tile_pool(name="rv", bufs=2))
        e_p = ctx.enter_context(tc.tile_pool(name="e", bufs=2))
        big_p = ctx.enter_context(tc.tile_pool(name="big", bufs=1))
        small = ctx.enter_context(tc.tile_pool(name="small", bufs=1))

        ones_sb = const_p.tile([128, 128], F32, tag="ones")
        nc.vector.memset(ones_sb[:], 1.0)
        ones_bf = const_p.tile([64, 512], BF16, tag="onesbf")
        nc.vector.memset(ones_bf[:], 1.0)
        magic_sb = const_p.tile([C, 1], I32, tag="magic")
        nc.vector.memset(magic_sb[:], MAGIC)
        gsim_sb = const_p.tile([1, 1], F32, tag="gsim")
        nc.sync.dma_start(out=gsim_sb[:], in_=gsim_d[:])
        gval_sb = const_p.tile([C, 1], F32, tag="gval")
        nc.sync.dma_start(out=gval_sb[:], in_=gval_d[:])
        bval_sb = const_p.tile([C, 1], F32, tag="bval")
        nc.sync.dma_start(out=bval_sb[:], in_=bval_d[:])

        s_bcast = small.tile([128, 1], F32, tag="sbc")

        # ---- phase 0: Gram-matrix logits variance -> s ----
        with tc.tile_pool(name="qt", bufs=2) as qt_p, tc.tile_pool(
            name="gram", bufs=2, space="PSUM"
        ) as gram_p, tc.tile_pool(name="sf", bufs=1, space="PSUM") as sf_p, tc.tile_pool(
            name="warm", bufs=1, space="PSUM"
        ) as warm_p:
            wps = warm_p.tile([128, 512], F32, tag="warm")
            for _ in range(14):
                nc.tensor.matmul(
                    wps[:], ones_bf[:, 0:128], ones_bf[:, :], start=True, stop=True
                )

            acc = small.tile([65, NB], F32, tag="acc")
            for b in range(NB):
                qt_sb = qt_p.tile([128, NCH, 65], BF16, tag="qt")
                nc.gpsimd.dma_start(out=qt_sb[:], in_=qt1_d[b])
                kt_sb = qt_p.tile([128, NCH, 65], BF16, tag="kt")
                nc.gpsimd.dma_start(out=kt_sb[:], in_=kt1_d[b])

                gq_ps = gram_p.tile([65, 65], F32, tag="gq")
                gk_ps = gram_p.tile([65, 65], F32, tag="gk")
                for c in range(NCH):
                    nc.tensor.matmul(
                        gq_ps[:],
                        qt_sb[:, c, :],
                        qt_sb[:, c, :],
                        start=(c == 0),
                        stop=(c == NCH - 1),
                    )
                for c in range(NCH):
                    nc.tensor.matmul(
                        gk_ps[:],
                        kt_sb[:, c, :],
                        kt_sb[:, c, :],
                        start=(c == 0),
                        stop=(c == NCH - 1),
                    )
                gq_sb = small.tile([65, 65], F32, tag="gq_sb")
                nc.vector.tensor_copy(gq_sb[:], gq_ps[:])
                gk_sb = small.tile([65, 65], F32, tag="gk_sb")
                nc.vector.tensor_copy(gk_sb[:], gk_ps[:])
                prod = small.tile([65, 65], F32, tag="prod")
                nc.vector.tensor_mul(prod[:], gq_sb[:], gk_sb[:])
                nc.vector.reduce_sum(
                    acc[:, b : b + 1], prod[:, 0:64], axis=mybir.AxisListType.X
                )

            red = small.tile([65, 1], F32, tag="red")
            nc.vector.reduce_sum(red[:], acc[:], axis=mybir.AxisListType.X)
            rhs65 = small.tile([65, 1], F32, tag="rhs65")
            nc.vector.tensor_scalar_mul(rhs65[0:64, :], red[0:64, :], 1.0 / NLM)
            nc.vector.tensor_scalar_mul(rhs65[64:65, :], red[64:65, :], 1.0 / NLM)
            nc.vector.tensor_mul(rhs65[64:65, :], rhs65[64:65, :], rhs65[64:65, :])
            nc.vector.tensor_scalar_mul(rhs65[64:65, :], rhs65[64:65, :], -1.0)
            var_ps = sf_p.tile([1, 1], F32, tag="var")
            nc.tensor.matmul(
                var_ps[:], ones_sb[0:65, 0:1], rhs65[:], start=True, stop=True
            )
            sv = small.tile([1, 6], F32, tag="sv")
            nc.vector.tensor_scalar_add(sv[:, 0:1], var_ps[:], EPS)
            _newton_rsqrt(nc, sv[:, 0:1], sv[:, 1:2], sv[:, 2:3], magic_sb[0:1, :])
            nc.vector.tensor_mul(sv[:, 3:4], sv[:, 1:2], gsim_sb[:])
            sb_ps = sf_p.tile([128, 1], F32, tag="sb")
            nc.tensor.matmul(
                sb_ps[:], ones_sb[0:1, 0:128], sv[:, 3:4], start=True, stop=True
            )
            nc.vector.tensor_copy(s_bcast[:], sb_ps[:])

        # ---- phase A: QK -> exp -> PV ----
        ue_sb = big_p.tile([C, NB, L], F32, tag="ue")
        stats = small.tile([C, NB * 2 * 6], F32, tag="stats")

        with tc.tile_pool(name="lg", bufs=2, space="PSUM") as lg_p, tc.tile_pool(
            name="pv", bufs=1, space="PSUM"
        ) as pv_p, tc.tile_pool(name="dn", bufs=1, space="PSUM") as dn_p:
            for b in range(NB):
                pair, r = divmod(b, 2)
                if r == 0:
                    q2_sb = q2_p.tile([128, L], BF16, tag="q2")
                    nc.gpsimd.dma_start(out=q2_sb[:], in_=q2_d[pair])
                    k2_sb = q2_p.tile([128, L], BF16, tag="k2")
                    nc.gpsimd.dma_start(out=k2_sb[:], in_=k2_d[pair])
                vt_sb = vt_p.tile([128, NCH, 65], BF16, tag="vt")
                nc.gpsimd.dma_start(out=vt_sb[:], in_=vt1_d[b])

                pv = pv_p.tile([65, L], F32, tag="pv")
                for c in range(NCH):
                    lg = lg_p.tile([128, L], F32, tag="lg")
                    for j in range(2):
                        nc.tensor.matmul(
                            lg[:, j * 512 : (j + 1) * 512],
                            k2_sb[r * 64 : r * 64 + 64, c * 128 : (c + 1) * 128],
                            q2_sb[r * 64 : r * 64 + 64, j * 512 : (j + 1) * 512],
                            start=True,
                            stop=True,
                        )
                    pt = pt_p.tile([128, L], BF16, tag="pt")
                    nc.scalar.activation(pt[:], lg[:], AF.Exp, scale=s_bcast[:, 0:1])
                    for j in range(2):
                        nc.tensor.matmul(
                            pv[:, j * 512 : (j + 1) * 512],
                            vt_sb[:, c, :],
                            pt[:, j * 512 : (j + 1) * 512],
                            start=(c == 0),
                            stop=(c == NCH - 1),
                            skip_group_check=True,
                        )
                rv_sb = rv_p.tile([65, L], F32, tag="rv")
                nc.vector.tensor_copy(rv_sb[:], pv[:])
                # broadcast denominators (row 64) down to 64 partitions via
                # a K=1 ones-matmul (ones row at partition 64)
                dn = dn_p.tile([64, L], F32, tag="dn")
                for j in range(2):
                    nc.tensor.matmul(
                        dn[:, j * 512 : (j + 1) * 512],
                        ones_sb[64:65, 0:64],
                        rv_sb[64:65, j * 512 : (j + 1) * 512],
                        start=True,
                        stop=True,
                    )
                e_sb = e_p.tile([64, L], F32, tag="e")
                nc.vector.reciprocal_approx_fast(out=e_sb[:], in_=dn[:])
                nc.vector.tensor_mul(ue_sb[:, b, :], rv_sb[0:64, :], e_sb[:])
                for half in range(2):
                    nc.vector.bn_stats(
                        stats[:, (b * 2 + half) * 6 : (b * 2 + half + 1) * 6],
                        ue_sb[:, b, half * 512 : (half + 1) * 512],
                    )

        # ---- phase B: val-BN affine + gelu + store ----
        chan = small.tile([C, 2], F32, tag="chan")
        nc.vector.bn_aggr(chan[:], stats[:])
        vb = small.tile([C, 6], F32, tag="vb")
        nc.vector.tensor_scalar_add(vb[:, 0:1], chan[:, 1:2], EPS)
        _newton_rsqrt(nc, vb[:, 0:1], vb[:, 1:2], vb[:, 2:3], magic_sb[:, :])
        a_c = small.tile([C, 1], F32, tag="a_c")
        nc.vector.tensor_mul(a_c[:], gval_sb[:], vb[:, 1:2])
        b_c = small.tile([C, 1], F32, tag="b_c")
        nc.vector.tensor_mul(vb[:, 3:4], chan[:, 0:1], a_c[:])
        nc.vector.tensor_sub(b_c[:], bval_sb[:], vb[:, 3:4])

        out_sb = big_p.tile([C, NB, L], F32, tag="outsb")
        nc.scalar.activation(
            out_sb[:], ue_sb[:], AF.Gelu, scale=a_c[:, 0:1], bias=b_c[:, 0:1]
        )
        for b in range(NB):
            nc.sync.dma_start(out=out_d[b], in_=out_sb[:, b, :])

    nc.compile()
    return nc


def make_in_map(q, k, v, gamma_sim, beta_sim, gamma_val, beta_val, h):
    """Build the per-core (per-head) input map. Layout-only host prep."""
    qh = np.ascontiguousarray(q[:, h])
    kh = np.ascontiguousarray(k[:, h])
    vh = np.ascontiguousarray(v[:, h])

    def t1(x):
        xt = x.transpose(0, 2, 1)
        out = np.ones((NB, L, 65), dtype=np.float32)
        out[:, :, :64] = xt
        return np.ascontiguousarray(
            out.reshape(NB, NCH, 128, 65).transpose(0, 2, 1, 3)
        )

    return {
        "q2": qh.reshape(NB // 2, 128, L),
        "k2": kh.reshape(NB // 2, 128, L),
        "qt1": t1(qh),
        "kt1": t1(kh),
        "vt1": t1(vh),
        "g_sim": np.asarray(gamma_sim[h], dtype=np.float32).reshape(1, 1),
        "gamma_val": np.asarray(
            gamma_val[h * C : (h + 1) * C], dtype=np.float32
        ).reshape(C, 1),
        "beta_val": np.asarray(
            beta_val[h * C : (h + 1) * C], dtype=np.float32
        ).reshape(C, 1),
    }



_CACHED_NC = None


def _setup_profiling():
    """Make run_bass_kernel_spmd(trace=True) work on images missing
    antenv.axon_hooks: inject the ctypes NTFF hook + keep artifacts local."""
    import contextlib
    import ctypes
    import types

    try:
        from antenv.axon_hooks import get_axon_ntff_profile_hook  # noqa: F401
    except ImportError:
        so_path = os.environ.get("AXON_PJRT_SO", "/opt/axon/libaxon_pjrt.so")
        lib = ctypes.CDLL(so_path)
        lib.axon_start_nrt_profile.argtypes = [
            ctypes.POINTER(ctypes.c_int64),
            ctypes.c_size_t,
        ]
        lib.axon_start_nrt_profile.restype = ctypes.c_int64
        lib.axon_stop_nrt_profile.argtypes = [ctypes.c_char_p]
        lib.axon_stop_nrt_profile.restype = ctypes.c_int64

        @contextlib.contextmanager
        def _hook(output_dir, device_ids):
            import jax

            jax.devices()
            if device_ids:
                ids = (ctypes.c_int64 * len(device_ids))(*device_ids)
                rc = lib.axon_start_nrt_profile(ids, len(device_ids))
            else:
                rc = lib.axon_start_nrt_profile(None, 0)
            if rc != 0:
                raise RuntimeError(f"axon_start_nrt_profile rc={rc}")
            try:
                yield
            finally:
                n = lib.axon_stop_nrt_profile(str(output_dir).encode())
                print(f"ntff profile: {n} file(s) -> {output_dir}", file=sys.stderr)

        mod = types.ModuleType("antenv.axon_hooks")
        mod.get_axon_ntff_profile_hook = lambda: _hook
        mod.set_axon_ntff_profile_hook = lambda h: None
        import antenv

        sys.modules["antenv.axon_hooks"] = mod
        antenv.axon_hooks = mod

    import concourse.bass_utils as bu

    bu.upload_artifacts = lambda tmpdir: f"local://{tmpdir}"


def kernel(query, key, value, gamma_sim, beta_sim, gamma_val, beta_val):
    global _CACHED_NC
    from concourse.bass_utils import run_bass_kernel_spmd

    query = np.asarray(query, dtype=np.float32)
    key = np.asarray(key, dtype=np.float32)
    value = np.asarray(value, dtype=np.float32)
    gamma_sim = np.asarray(gamma_sim, dtype=np.float32)
    gamma_val = np.asarray(gamma_val, dtype=np.float32)
    beta_val = np.asarray(beta_val, dtype=np.float32)

    if _CACHED_NC is None:
        _CACHED_NC = build_nc()
    nc = _CACHED_NC

    in_maps = [
        make_in_map(query, key, value, gamma_sim, None, gamma_val, beta_val, h)
        for h in range(8)
    ]
    trace = bool(int(os.environ.get("BASS_PROFILE", "0")))
    tmpdir = os.environ.get("BASS_PROFILE_DIR") or None
    if trace:
        try:
            _setup_profiling()
        except Exception as e:  # noqa: BLE001
            print(f"profiling setup failed ({e}); running untraced", file=sys.stderr)
            trace = False
    try:
        res = run_bass_kernel_spmd(
            nc, in_maps, list(range(8)), trace=trace, tmpdir=tmpdir
        )
    except Exception:
        if not trace:
            raise
        print("traced run failed; retrying untraced", file=sys.stderr)
        res = run_bass_kernel_spmd(nc, in_maps, list(range(8)), trace=False)
    if trace and res.exec_time_ns is not None:
        print(f"HW exec time: {res.exec_time_ns} ns")

    out = np.empty((NB, 8 * C, L), dtype=np.float32)
    for h in range(8):
        out[:, h * C : (h + 1) * C, :] = res.results[h]["out"]
    return out

